# revision 17
# baseline (speedup 1.0000x reference)
"""Multi-head attention (B=2, S=2048, D=1024, H=16, causal) on 8 trn2 cores.

Sharding: core c -> batch b=c//4, head-group g=c%4 (4 heads, d-slice of 256).
Each core computes, for its batch and its 4 heads:
  Q/K/V projections (from host-pre-transposed activations, bf16),
  causal attention in transposed layout (logits^T blocks [key128 x query512]),
  softmax via exp on ScalarE (no max-subtraction needed: |logits|<~3),
  PV with ones-augmented V (row 64 = softmax denominator Z),
  normalization via reciprocal + PE-broadcast,
  partial output projection -> out_t[e, s] (partial over d-slice).
Host sums the 4 partials per batch and transposes.
"""

import os
import sys

sys.path.insert(0, "/opt/trn_rl_repo")

import numpy as np
import ml_dtypes

import concourse.bass as bass
import concourse.bacc as bacc
import concourse.mybir as mybir
import concourse.tile as tile

F32 = mybir.dt.float32
F32R = mybir.dt.float32r
BF16 = mybir.dt.bfloat16
BF16NP = ml_dtypes.bfloat16
Exp = mybir.ActivationFunctionType.Exp

B, S, D, H = 2, 2048, 1024, 16
HD = 64          # head dim
DL = 256         # local d (4 heads) per core
NE = 8           # e-chunks of 128 in model dim
SJ = 128         # key block
SI = 512         # query block (one psum bank of f32)
NSI = S // SI    # 4
NSJ = S // SJ    # 16
SCALE = 0.125    # 1/sqrt(64)


def build_nc(repeat=1):
    import contextlib
    nc = bacc.Bacc(None, target_bir_lowering=False)

    xq = nc.dram_tensor("xq", [D, S], BF16, kind="ExternalInput")   # q[b].T
    xk = nc.dram_tensor("xk", [D, S], BF16, kind="ExternalInput")
    xv = nc.dram_tensor("xv", [D, S], BF16, kind="ExternalInput")
    wq = nc.dram_tensor("wq", [D, DL], BF16, kind="ExternalInput")  # w_q[gsl].T
    wk = nc.dram_tensor("wk", [D, DL], BF16, kind="ExternalInput")
    wv = nc.dram_tensor("wv", [D, DL], BF16, kind="ExternalInput")
    wo = nc.dram_tensor("wo", [DL, D], BF16, kind="ExternalInput")  # w_o[:,gsl].T
    stair = nc.dram_tensor("stair", [128, 1024], BF16, kind="ExternalInput")
    out_t = nc.dram_tensor("out_t", [D, S], F32, kind="ExternalOutput")

    with tile.TileContext(nc) as tc, \
            tc.tile_pool(name="consts", bufs=1) as consts, \
            tc.tile_pool(name="xin", bufs=10) as xinp, \
            tc.tile_pool(name="qkt", bufs=1) as qktp, \
            tc.tile_pool(name="vaug", bufs=1) as vaugp, \
            tc.tile_pool(name="psb", bufs=3) as psbp, \
            tc.tile_pool(name="attn_sb", bufs=3) as attnsbp, \
            tc.tile_pool(name="small", bufs=4) as smallp, \
            tc.tile_pool(name="qk_ps", bufs=2, space="PSUM") as qkps, \
            tc.tile_pool(name="att_ps", bufs=2, space="PSUM") as attps, \
            tc.tile_pool(name="mix_ps", bufs=2, space="PSUM") as mixps:

        # ---- constants ----
        wq_sb = consts.tile([128, NE, DL], BF16)
        wk_sb = consts.tile([128, NE, DL], BF16)
        wv_sb = consts.tile([128, NE, DL], BF16)
        wo_sb = consts.tile([128, 2, D], BF16)
        stair_sb = consts.tile([128, 1024], BF16)
        ones_sb = consts.tile([128, 64], F32)
        for t, dram in ((wq_sb, wq), (wk_sb, wk), (wv_sb, wv)):
            nc.sync.dma_start(out=t, in_=dram.rearrange("(c p) d -> p c d", p=128))
        nc.sync.dma_start(out=wo_sb, in_=wo.rearrange("(t p) e -> p t e", p=128))
        nc.sync.dma_start(out=stair_sb, in_=stair[:, :])
        nc.vector.memset(ones_sb, 1.0)

        rep_ctx = tc.For_i(0, repeat, 1) if repeat > 1 else contextlib.nullcontext()
        with rep_ctx:
            _build_body(nc, tc, locals())
    nc.compile()
    return nc


def _build_body(nc, tc, env):
    (xq, xk, xv, wo, out_t) = (env[n] for n in ("xq", "xk", "xv", "wo", "out_t"))
    (wq_sb, wk_sb, wv_sb, wo_sb, stair_sb, ones_sb) = (
        env[n] for n in ("wq_sb", "wk_sb", "wv_sb", "wo_sb", "stair_sb", "ones_sb"))
    (xinp, qktp, vaugp, psbp, attnsbp, smallp, qkps, attps, mixps) = (
        env[n] for n in ("xinp", "qktp", "vaugp", "psbp", "attnsbp", "smallp",
                         "qkps", "attps", "mixps"))
    if True:

        # ---- projections ----
        # xin tiles: [128, NE, S] view of x{q,k,v}^T; each e-chunk is [128, S]
        qt_sb = [qktp.tile([128, S], BF16, name=f"qt{p}", tag=f"qt{p}")
                 for p in range(2)]
        kt_sb = [qktp.tile([128, S], BF16, name=f"kt{p}", tag=f"kt{p}")
                 for p in range(2)]
        # v_aug[sjb]: [128, 4*65]; per head h: cols h*65..h*65+63 = V, col h*65+64 = 1
        v_aug = [vaugp.tile([128, 4 * 65], BF16, name=f"vaug{j}", tag=f"vaug{j}")
                 for j in range(NSJ)]

        def load_x(dram, name):
            """8 chunk tiles [128, S], one DMA each (single producer per tile)."""
            src = dram.rearrange("(c p) s -> p c s", p=128)
            xt = []
            for cc in range(NE):
                t = xinp.tile([128, S], BF16, name=f"{name}{cc}", tag="xin")
                nc.sync.dma_start(out=t, in_=src[:, cc, :])
                xt.append(t)
            return xt

        # Q and K projections -> transposed layout [d_local(128/pair), s]
        for dram, wsb, dst, nm in ((xq, wq_sb, qt_sb, "xq_sb"),
                                   (xk, wk_sb, kt_sb, "xk_sb")):
            xt = load_x(dram, nm)
            for pr in range(2):          # d-tile == head pair
                for sr in range(2):      # s range of 1024
                    ps = qkps.tile([128, 1024], F32, name="proj_ps", tag="qk")
                    for half in range(2):
                        for e in range(NE):
                            nc.tensor.matmul(
                                ps[:, half * SI:(half + 1) * SI],
                                lhsT=wsb[:, e, pr * 128:(pr + 1) * 128],
                                rhs=xt[e][:, sr * 1024 + half * SI:
                                          sr * 1024 + half * SI + SI],
                                start=(e == 0), stop=(e == NE - 1),
                                skip_group_check=True)
                    nc.any.tensor_copy(
                        out=dst[pr][:, sr * 1024:(sr + 1) * 1024], in_=ps)

        # V projection -> [s(128-blocks), d_local 256], interleaved with ones
        xtv = load_x(xv, "xv_sb")
        for sjb in range(NSJ):
            ps = mixps.tile([128, DL], F32, name="vproj_ps", tag="mix")
            for e in range(NE):
                nc.tensor.matmul(
                    ps,
                    lhsT=xtv[e][:, sjb * SJ:(sjb + 1) * SJ],
                    rhs=wv_sb[:, e, :],
                    start=(e == 0), stop=(e == NE - 1),
                    skip_group_check=True)
            va = v_aug[sjb]
            va_v = va.rearrange("p (h c) -> p h c", h=4)
            nc.any.tensor_copy(
                out=va_v[:, :, 0:64],
                in_=ps.rearrange("p (h c) -> p h c", h=4))
            nc.vector.memset(va_v[:, :, 64:65], 1.0)

        # ---- attention + output projection, per query block of 512 ----
        for si in range(NSI):
            attn_pair = [attnsbp.tile([128, SI], BF16, name=f"ap{pr}",
                                      tag=f"ap{pr}") for pr in range(2)]
            for pr in range(2):
                nsj = (si + 1) * 4
                att = [attps.tile([65, SI], F32, name=f"att{h2}", tag="att")
                       for h2 in range(2)]
                for sjb in range(nsj):
                    a = sjb * SJ - si * SI     # >0 and <512 only on diagonal
                    amax = max(0, a)
                    qk = qkps.tile([128, 2, SI], F32, name="qk_ps", tag="qk")
                    for h2 in range(2):
                        nc.tensor.matmul(
                            qk[:, h2, amax:SI],
                            lhsT=kt_sb[pr][h2 * 64:(h2 + 1) * 64,
                                           sjb * SJ:(sjb + 1) * SJ],
                            rhs=qt_sb[pr][h2 * 64:(h2 + 1) * 64,
                                          si * SI + amax:(si + 1) * SI],
                            start=True, stop=True, skip_group_check=True)
                    p_sb = psbp.tile([128, 2, SI], BF16, name="p_sb", tag="p")
                    nc.scalar.activation(
                        out=p_sb[:, :, amax:SI], in_=qk[:, :, amax:SI],
                        func=Exp, scale=SCALE)
                    if a >= 0:  # diagonal block: tril mask on cols [amax, amax+128)
                        for h2 in range(2):
                            nc.vector.tensor_mul(
                                p_sb[:, h2, amax:amax + SJ],
                                p_sb[:, h2, amax:amax + SJ],
                                stair_sb[:, 512:640])
                    for h2 in range(2):
                        h = pr * 2 + h2
                        nc.tensor.matmul(
                            att[h2][:, amax:SI],
                            lhsT=v_aug[sjb][:, h * 65:h * 65 + 65],
                            rhs=p_sb[:, h2, amax:SI],
                            start=(sjb == 0), stop=(sjb == nsj - 1),
                            skip_group_check=True)
                # normalize: attn_norm[d, s] = att[d, s] / att[64, s]
                for h2 in range(2):
                    rz = smallp.tile([128, SI], F32, name="rz", tag="rz")
                    nc.vector.reciprocal(rz[64:65, :], att[h2][64:65, :])
                    bc_sb = smallp.tile([64, SI], F32, name="bc_sb", tag="bcs")
                    nc.gpsimd.partition_broadcast(bc_sb, rz[64:65, :])
                    if h2 == 0:
                        nc.vector.tensor_mul(
                            attn_pair[pr][0:64, :], att[h2][0:64, :], bc_sb)
                    else:
                        tmp = smallp.tile([64, SI], BF16, name="tmp", tag="tmp")
                        nc.vector.tensor_mul(tmp, att[h2][0:64, :], bc_sb)
                        nc.sync.dma_start(out=attn_pair[pr][64:128, :], in_=tmp)
            # output projection for this query block
            for e in range(NE):
                po = mixps.tile([128, SI], F32, name="po", tag="mix")
                for t in range(2):
                    nc.tensor.matmul(
                        po,
                        lhsT=wo_sb[:, t, e * 128:(e + 1) * 128],
                        rhs=attn_pair[t],
                        start=(t == 0), stop=(t == 1), skip_group_check=True)
                po_sb = attnsbp.tile([128, SI], F32, name="po_sb", tag="po_sb")
                nc.vector.tensor_copy(out=po_sb, in_=po)
                nc.sync.dma_start(
                    out=out_t[e * 128:(e + 1) * 128, si * SI:(si + 1) * SI],
                    in_=po_sb)


def make_stair():
    p = np.arange(128)[:, None]
    x = np.arange(1024)[None, :]
    return (x >= p + 512).astype(BF16NP)


def host_prep(q, k, v, w_q, w_k, w_v, w_o):
    """Build the 8 per-core input maps."""
    stair = make_stair()
    xt = {}
    for b in range(B):
        xt[b] = [np.ascontiguousarray(a[b].T).astype(BF16NP) for a in (q, k, v)]
    maps = []
    for c in range(8):
        b, g = c // 4, c % 4
        sl = slice(g * DL, (g + 1) * DL)
        maps.append({
            "xq": xt[b][0], "xk": xt[b][1], "xv": xt[b][2],
            "wq": np.ascontiguousarray(w_q[sl].T).astype(BF16NP),
            "wk": np.ascontiguousarray(w_k[sl].T).astype(BF16NP),
            "wv": np.ascontiguousarray(w_v[sl].T).astype(BF16NP),
            "wo": np.ascontiguousarray(w_o[:, sl].T).astype(BF16NP),
            "stair": stair,
        })
    return maps


def kernel(q, k, v, mask, w_q, w_k, w_v, w_o):
    from concourse.bass_utils import run_bass_kernel_spmd

    q, k, v = (np.asarray(a, np.float32) for a in (q, k, v))
    w_q, w_k, w_v, w_o = (np.asarray(a, np.float32)
                          for a in (w_q, w_k, w_v, w_o))
    nc = build_nc()
    maps = host_prep(q, k, v, w_q, w_k, w_v, w_o)
    trace = bool(int(os.environ.get("KERNEL_TRACE", "0")))
    res = run_bass_kernel_spmd(nc, maps, list(range(8)), trace=trace)
    if trace:
        kernel.last_results = res
    out = np.empty((B, S, D), np.float32)
    for b in range(B):
        acc = res.results[b * 4]["out_t"].astype(np.float32)
        for g in range(1, 4):
            acc = acc + res.results[b * 4 + g]["out_t"]
        out[b] = acc.T
    return out


# revision 22
# speedup vs baseline: 1.2373x; 1.2373x over previous
"""Multi-head attention (B=2, S=2048, D=1024, H=16, causal) on 8 trn2 cores.

Sharding: core c -> batch b=c//4, head-group g=c%4 (4 heads, d-slice of 256).
Each core computes, for its batch and its 4 heads:
  Q/K/V projections (from host-pre-transposed activations, bf16),
  causal attention in transposed layout (logits^T blocks [key128 x query512]),
  softmax via exp on ScalarE (no max-subtraction needed: |logits|<~3),
  PV with ones-augmented V (row 64 = softmax denominator Z),
  normalization via reciprocal + PE-broadcast,
  partial output projection -> out_t[e, s] (partial over d-slice).
Host sums the 4 partials per batch and transposes.
"""

import os
import sys

sys.path.insert(0, "/opt/trn_rl_repo")

import numpy as np
import ml_dtypes

import concourse.bass as bass
import concourse.bacc as bacc
import concourse.mybir as mybir
import concourse.tile as tile

F32 = mybir.dt.float32
F32R = mybir.dt.float32r
BF16 = mybir.dt.bfloat16
BF16NP = ml_dtypes.bfloat16
Exp = mybir.ActivationFunctionType.Exp

B, S, D, H = 2, 2048, 1024, 16
HD = 64          # head dim
DL = 256         # local d (4 heads) per core
NE = 8           # e-chunks of 128 in model dim
SJ = 128         # key block
SI = 512         # query block (one psum bank of f32)
NSI = S // SI    # 4
NSJ = S // SJ    # 16
SCALE = 0.125    # 1/sqrt(64)


def build_nc(repeat=1, parts="all"):
    import contextlib
    nc = bacc.Bacc(None, target_bir_lowering=False)

    xq = nc.dram_tensor("xq", [D, S], BF16, kind="ExternalInput")   # q[b].T
    xk = nc.dram_tensor("xk", [D, S], BF16, kind="ExternalInput")
    xv = nc.dram_tensor("xv", [D, S], BF16, kind="ExternalInput")
    wq = nc.dram_tensor("wq", [D, DL], BF16, kind="ExternalInput")  # w_q[gsl].T
    wk = nc.dram_tensor("wk", [D, DL], BF16, kind="ExternalInput")
    wv = nc.dram_tensor("wv", [D, DL], BF16, kind="ExternalInput")
    wo = nc.dram_tensor("wo", [DL, D], BF16, kind="ExternalInput")  # w_o[:,gsl].T
    stair = nc.dram_tensor("stair", [128, 1024], BF16, kind="ExternalInput")
    out_t = nc.dram_tensor("out_t", [D, S], F32, kind="ExternalOutput")

    with tile.TileContext(nc) as tc, \
            tc.tile_pool(name="consts", bufs=1) as consts, \
            tc.tile_pool(name="xin", bufs=10) as xinp, \
            tc.tile_pool(name="qkt", bufs=1) as qktp, \
            tc.tile_pool(name="vaug", bufs=1) as vaugp, \
            tc.tile_pool(name="psb", bufs=3) as psbp, \
            tc.tile_pool(name="attn_sb", bufs=3) as attnsbp, \
            tc.tile_pool(name="small", bufs=4) as smallp, \
            tc.tile_pool(name="qk_ps", bufs=2, space="PSUM") as qkps, \
            tc.tile_pool(name="att_ps", bufs=2, space="PSUM") as attps, \
            tc.tile_pool(name="mix_ps", bufs=2, space="PSUM") as mixps:

        # ---- constants ----
        wq_sb = consts.tile([128, NE, DL], BF16)
        wk_sb = consts.tile([128, NE, DL], BF16)
        wv_sb = consts.tile([128, NE, DL], BF16)
        wo_sb = consts.tile([128, 2, D], BF16)
        stair_sb = consts.tile([128, 1024], BF16)
        ones_sb = consts.tile([128, 64], F32)
        for t, dram in ((wq_sb, wq), (wk_sb, wk), (wv_sb, wv)):
            nc.sync.dma_start(out=t, in_=dram.rearrange("(c p) d -> p c d", p=128))
        nc.sync.dma_start(out=wo_sb, in_=wo.rearrange("(t p) e -> p t e", p=128))
        nc.sync.dma_start(out=stair_sb, in_=stair[:, :])
        nc.vector.memset(ones_sb, 1.0)

        rep_ctx = tc.For_i(0, repeat, 1) if repeat > 1 else contextlib.nullcontext()
        with rep_ctx:
            _build_body(nc, tc, locals(), parts)
    nc.compile()
    return nc


def _build_body(nc, tc, env, parts="all"):
    (xq, xk, xv, wo, out_t) = (env[n] for n in ("xq", "xk", "xv", "wo", "out_t"))
    (wq_sb, wk_sb, wv_sb, wo_sb, stair_sb, ones_sb) = (
        env[n] for n in ("wq_sb", "wk_sb", "wv_sb", "wo_sb", "stair_sb", "ones_sb"))
    (xinp, qktp, vaugp, psbp, attnsbp, smallp, qkps, attps, mixps) = (
        env[n] for n in ("xinp", "qktp", "vaugp", "psbp", "attnsbp", "smallp",
                         "qkps", "attps", "mixps"))
    if True:

        # ---- projections ----
        # xin tiles: [128, NE, S] view of x{q,k,v}^T; each e-chunk is [128, S]
        qt_sb = [qktp.tile([128, S], BF16, name=f"qt{p}", tag=f"qt{p}")
                 for p in range(2)]
        kt_sb = [qktp.tile([128, S], BF16, name=f"kt{p}", tag=f"kt{p}")
                 for p in range(2)]
        # v_aug[sjb]: [128, 4*65]; per head h: cols h*65..h*65+63 = V, col h*65+64 = 1
        v_aug = [vaugp.tile([128, 4 * 65], BF16, name=f"vaug{j}", tag=f"vaug{j}")
                 for j in range(NSJ)]

        def load_x(dram, name):
            """8 chunk tiles [128, S], one DMA each (single producer per tile)."""
            src = dram.rearrange("(c p) s -> p c s", p=128)
            xt = []
            for cc in range(NE):
                t = xinp.tile([128, S], BF16, name=f"{name}{cc}", tag="xin")
                nc.sync.dma_start(out=t, in_=src[:, cc, :])
                xt.append(t)
            return xt

        # Q and K projections -> transposed layout [d_local(128/pair), s]
        proj_list = (((xq, wq_sb, qt_sb, "xq_sb"),
                      (xk, wk_sb, kt_sb, "xk_sb"))
                     if parts in ("all", "proj") else ())
        for dram, wsb, dst, nm in proj_list:
            xt = load_x(dram, nm)
            for pr in range(2):          # d-tile == head pair
                for sr in range(2):      # s range of 1024
                    ps = qkps.tile([128, 1024], F32, name="proj_ps", tag="qk")
                    for half in range(2):
                        for e in range(NE):
                            nc.tensor.matmul(
                                ps[:, half * SI:(half + 1) * SI],
                                lhsT=wsb[:, e, pr * 128:(pr + 1) * 128],
                                rhs=xt[e][:, sr * 1024 + half * SI:
                                          sr * 1024 + half * SI + SI],
                                start=(e == 0), stop=(e == NE - 1),
                                skip_group_check=True)
                    nc.any.tensor_copy(
                        out=dst[pr][:, sr * 1024:(sr + 1) * 1024], in_=ps)

        # V projection -> [s(128-blocks), d_local 256], interleaved with ones
        xtv = load_x(xv, "xv_sb") if parts in ("all", "proj") else None
        for sjb in (range(NSJ) if xtv is not None else ()):
            ps = mixps.tile([128, DL], F32, name="vproj_ps", tag="mix")
            for e in range(NE):
                nc.tensor.matmul(
                    ps,
                    lhsT=xtv[e][:, sjb * SJ:(sjb + 1) * SJ],
                    rhs=wv_sb[:, e, :],
                    start=(e == 0), stop=(e == NE - 1),
                    skip_group_check=True)
            va = v_aug[sjb]
            va_v = va.rearrange("p (h c) -> p h c", h=4)
            nc.any.tensor_copy(
                out=va_v[:, :, 0:64],
                in_=ps.rearrange("p (h c) -> p h c", h=4))
            nc.vector.memset(va_v[:, :, 64:65], 1.0)

        if parts == "attn":   # bench-only: fake projection outputs
            for t in qt_sb + kt_sb:
                nc.vector.memset(t, 0.5)
            for t in v_aug:
                nc.vector.memset(t, 0.5)

        # ---- attention + output projection, per query block of 512 ----
        for si in (range(NSI) if parts in ("all", "attn") else ()):
            attn_pair = [attnsbp.tile([128, SI], BF16, name=f"ap{pr}",
                                      tag=f"ap{pr}") for pr in range(2)]
            for pr in range(2):
                nsj = (si + 1) * 4
                att = [attps.tile([65, SI], F32, name=f"att{h2}", tag="att")
                       for h2 in range(2)]
                for sjb in range(nsj):
                    a = sjb * SJ - si * SI     # >0 and <512 only on diagonal
                    amax = max(0, a)
                    qk = qkps.tile([128, 2, SI], F32, name="qk_ps", tag="qk")
                    for h2 in range(2):
                        nc.tensor.matmul(
                            qk[:, h2, amax:SI],
                            lhsT=kt_sb[pr][h2 * 64:(h2 + 1) * 64,
                                           sjb * SJ:(sjb + 1) * SJ],
                            rhs=qt_sb[pr][h2 * 64:(h2 + 1) * 64,
                                          si * SI + amax:(si + 1) * SI],
                            start=True, stop=True, skip_group_check=True)
                    p_sb = psbp.tile([128, 2, SI], BF16, name="p_sb", tag="p")
                    nc.scalar.activation(
                        out=p_sb[:, :, amax:SI], in_=qk[:, :, amax:SI],
                        func=Exp, scale=SCALE)
                    if a >= 0:  # diagonal block: tril mask on cols [amax, amax+128)
                        for h2 in range(2):
                            nc.vector.tensor_mul(
                                p_sb[:, h2, amax:amax + SJ],
                                p_sb[:, h2, amax:amax + SJ],
                                stair_sb[:, 512:640])
                    for h2 in range(2):
                        h = pr * 2 + h2
                        nc.tensor.matmul(
                            att[h2][:, amax:SI],
                            lhsT=v_aug[sjb][:, h * 65:h * 65 + 65],
                            rhs=p_sb[:, h2, amax:SI],
                            start=(sjb == 0), stop=(sjb == nsj - 1),
                            skip_group_check=True)
                # normalize: attn_norm[d, s] = att[d, s] / att[64, s]
                for h2 in range(2):
                    rz = smallp.tile([128, SI], F32, name="rz", tag="rz")
                    nc.vector.reciprocal(rz[64:65, :], att[h2][64:65, :])
                    bc_sb = smallp.tile([64, SI], F32, name="bc_sb", tag="bcs")
                    nc.gpsimd.partition_broadcast(bc_sb, rz[64:65, :])
                    if h2 == 0:
                        nc.vector.tensor_mul(
                            attn_pair[pr][0:64, :], att[h2][0:64, :], bc_sb)
                    else:
                        tmp = smallp.tile([64, SI], BF16, name="tmp", tag="tmp")
                        nc.vector.tensor_mul(tmp, att[h2][0:64, :], bc_sb)
                        nc.sync.dma_start(out=attn_pair[pr][64:128, :], in_=tmp)
            # output projection for this query block
            for e in range(NE):
                po = mixps.tile([128, SI], F32, name="po", tag="mix")
                for t in range(2):
                    nc.tensor.matmul(
                        po,
                        lhsT=wo_sb[:, t, e * 128:(e + 1) * 128],
                        rhs=attn_pair[t],
                        start=(t == 0), stop=(t == 1), skip_group_check=True)
                po_sb = attnsbp.tile([128, SI], F32, name="po_sb", tag="po_sb")
                nc.vector.tensor_copy(out=po_sb, in_=po)
                nc.sync.dma_start(
                    out=out_t[e * 128:(e + 1) * 128, si * SI:(si + 1) * SI],
                    in_=po_sb)


def make_stair():
    p = np.arange(128)[:, None]
    x = np.arange(1024)[None, :]
    return (x >= p + 512).astype(BF16NP)


def host_prep(q, k, v, w_q, w_k, w_v, w_o):
    """Build the 8 per-core input maps."""
    stair = make_stair()
    xt = {}
    for b in range(B):
        xt[b] = [np.ascontiguousarray(a[b].T).astype(BF16NP) for a in (q, k, v)]
    maps = []
    for c in range(8):
        b, g = c // 4, c % 4
        sl = slice(g * DL, (g + 1) * DL)
        maps.append({
            "xq": xt[b][0], "xk": xt[b][1], "xv": xt[b][2],
            "wq": np.ascontiguousarray(w_q[sl].T).astype(BF16NP),
            "wk": np.ascontiguousarray(w_k[sl].T).astype(BF16NP),
            "wv": np.ascontiguousarray(w_v[sl].T).astype(BF16NP),
            "wo": np.ascontiguousarray(w_o[:, sl].T).astype(BF16NP),
            "stair": stair,
        })
    return maps


def kernel(q, k, v, mask, w_q, w_k, w_v, w_o):
    from concourse.bass_utils import run_bass_kernel_spmd

    q, k, v = (np.asarray(a, np.float32) for a in (q, k, v))
    w_q, w_k, w_v, w_o = (np.asarray(a, np.float32)
                          for a in (w_q, w_k, w_v, w_o))
    nc = build_nc()
    maps = host_prep(q, k, v, w_q, w_k, w_v, w_o)
    trace = bool(int(os.environ.get("KERNEL_TRACE", "0")))
    res = run_bass_kernel_spmd(nc, maps, list(range(8)), trace=trace)
    if trace:
        kernel.last_results = res
    out = np.empty((B, S, D), np.float32)
    for b in range(B):
        acc = res.results[b * 4]["out_t"].astype(np.float32)
        for g in range(1, 4):
            acc = acc + res.results[b * 4 + g]["out_t"]
        out[b] = acc.T
    return out


# revision 24
# speedup vs baseline: 1.5892x; 1.2844x over previous
"""Multi-head attention (B=2, S=2048, D=1024, H=16, causal) on 8 trn2 cores.

Sharding: core c -> batch b=c//4, head-group g=c%4 (4 heads, d-slice of 256).
Each core computes, for its batch and its 4 heads:
  Q/K/V projections (from host-pre-transposed activations, bf16),
  causal attention in transposed layout (logits^T blocks [key128 x query512]),
  softmax via exp on ScalarE (no max-subtraction needed: |logits|<~3),
  PV with ones-augmented V (row 64 = softmax denominator Z),
  normalization via reciprocal + PE-broadcast,
  partial output projection -> out_t[e, s] (partial over d-slice).
Host sums the 4 partials per batch and transposes.
"""

import os
import sys

sys.path.insert(0, "/opt/trn_rl_repo")

import numpy as np
import ml_dtypes

import concourse.bass as bass
import concourse.bacc as bacc
import concourse.mybir as mybir
import concourse.tile as tile

F32 = mybir.dt.float32
F32R = mybir.dt.float32r
BF16 = mybir.dt.bfloat16
BF16NP = ml_dtypes.bfloat16
Exp = mybir.ActivationFunctionType.Exp

B, S, D, H = 2, 2048, 1024, 16
HD = 64          # head dim
DL = 256         # local d (4 heads) per core
NE = 8           # e-chunks of 128 in model dim
SJ = 128         # key block
SI = 512         # query block (one psum bank of f32)
NSI = S // SI    # 4
NSJ = S // SJ    # 16
SCALE = 0.125    # 1/sqrt(64)


def build_nc(repeat=1, parts="all"):
    import contextlib
    nc = bacc.Bacc(None, target_bir_lowering=False)

    xq = nc.dram_tensor("xq", [D, S], BF16, kind="ExternalInput")   # q[b].T
    xk = nc.dram_tensor("xk", [D, S], BF16, kind="ExternalInput")
    xv = nc.dram_tensor("xv", [D, S], BF16, kind="ExternalInput")
    wq = nc.dram_tensor("wq", [D, DL], BF16, kind="ExternalInput")  # w_q[gsl].T
    wk = nc.dram_tensor("wk", [D, DL], BF16, kind="ExternalInput")
    wv = nc.dram_tensor("wv", [D, DL], BF16, kind="ExternalInput")
    wo = nc.dram_tensor("wo", [DL, D], BF16, kind="ExternalInput")  # w_o[:,gsl].T
    stair = nc.dram_tensor("stair", [128, 1024], BF16, kind="ExternalInput")
    out_t = nc.dram_tensor("out_t", [D, S], F32, kind="ExternalOutput")

    with tile.TileContext(nc) as tc, \
            tc.tile_pool(name="consts", bufs=1) as consts, \
            tc.tile_pool(name="xin", bufs=10) as xinp, \
            tc.tile_pool(name="qkt", bufs=1) as qktp, \
            tc.tile_pool(name="vaug", bufs=1) as vaugp, \
            tc.tile_pool(name="psb", bufs=3) as psbp, \
            tc.tile_pool(name="attn_sb", bufs=3) as attnsbp, \
            tc.tile_pool(name="small", bufs=4) as smallp, \
            tc.tile_pool(name="qk_ps", bufs=2, space="PSUM") as qkps, \
            tc.tile_pool(name="att_ps", bufs=2, space="PSUM") as attps, \
            tc.tile_pool(name="mix_ps", bufs=2, space="PSUM") as mixps:

        # ---- constants ----
        wq_sb = consts.tile([128, NE, DL], BF16)
        wk_sb = consts.tile([128, NE, DL], BF16)
        wv_sb = consts.tile([128, NE, DL], BF16)
        wo_sb = consts.tile([128, 2, D], BF16)
        stair_sb = consts.tile([128, 1024], BF16)
        ones_sb = consts.tile([128, 64], F32)
        for t, dram in ((wq_sb, wq), (wk_sb, wk), (wv_sb, wv)):
            nc.sync.dma_start(out=t, in_=dram.rearrange("(c p) d -> p c d", p=128))
        nc.sync.dma_start(out=wo_sb, in_=wo.rearrange("(t p) e -> p t e", p=128))
        nc.sync.dma_start(out=stair_sb, in_=stair[:, :])
        nc.vector.memset(ones_sb, 1.0)

        rep_ctx = tc.For_i(0, repeat, 1) if repeat > 1 else contextlib.nullcontext()
        with rep_ctx:
            _build_body(nc, tc, locals(), parts)
    nc.compile()
    return nc


def _build_body(nc, tc, env, parts="all"):
    (xq, xk, xv, wo, out_t) = (env[n] for n in ("xq", "xk", "xv", "wo", "out_t"))
    (wq_sb, wk_sb, wv_sb, wo_sb, stair_sb, ones_sb) = (
        env[n] for n in ("wq_sb", "wk_sb", "wv_sb", "wo_sb", "stair_sb", "ones_sb"))
    (xinp, qktp, vaugp, psbp, attnsbp, smallp, qkps, attps, mixps) = (
        env[n] for n in ("xinp", "qktp", "vaugp", "psbp", "attnsbp", "smallp",
                         "qkps", "attps", "mixps"))
    if True:

        # ---- projections ----
        # xin tiles: [128, NE, S] view of x{q,k,v}^T; each e-chunk is [128, S]
        qt_sb = [qktp.tile([128, S], BF16, name=f"qt{p}", tag=f"qt{p}")
                 for p in range(2)]
        kt_sb = [qktp.tile([128, S], BF16, name=f"kt{p}", tag=f"kt{p}")
                 for p in range(2)]
        # v_aug[sjb]: [128, 4*65]; per head h: cols h*65..h*65+63 = V, col h*65+64 = 1
        v_aug = [vaugp.tile([128, 4 * 65], BF16, name=f"vaug{j}", tag=f"vaug{j}")
                 for j in range(NSJ)]

        def load_x(dram, name):
            """8 chunk tiles [128, S], one DMA each (single producer per tile)."""
            src = dram.rearrange("(c p) s -> p c s", p=128)
            xt = []
            for cc in range(NE):
                t = xinp.tile([128, S], BF16, name=f"{name}{cc}", tag="xin")
                nc.sync.dma_start(out=t, in_=src[:, cc, :])
                xt.append(t)
            return xt

        # Q and K projections -> transposed layout [d_local(128/pair), s]
        proj_list = (((xq, wq_sb, qt_sb, "xq_sb"),
                      (xk, wk_sb, kt_sb, "xk_sb"))
                     if parts in ("all", "proj") else ())
        for dram, wsb, dst, nm in proj_list:
            xt = load_x(dram, nm)
            for pr in range(2):          # d-tile == head pair
                for sr in range(2):      # s range of 1024
                    ps = qkps.tile([128, 1024], F32, name="proj_ps", tag="qk")
                    for half in range(2):
                        for e in range(NE):
                            nc.tensor.matmul(
                                ps[:, half * SI:(half + 1) * SI],
                                lhsT=wsb[:, e, pr * 128:(pr + 1) * 128],
                                rhs=xt[e][:, sr * 1024 + half * SI:
                                          sr * 1024 + half * SI + SI],
                                start=(e == 0), stop=(e == NE - 1),
                                skip_group_check=True)
                    nc.any.tensor_copy(
                        out=dst[pr][:, sr * 1024:(sr + 1) * 1024], in_=ps)

        # V projection -> [s(128-blocks), d_local 256], interleaved with ones
        xtv = load_x(xv, "xv_sb") if parts in ("all", "proj") else None
        for sjb in (range(NSJ) if xtv is not None else ()):
            ps = mixps.tile([128, DL], F32, name="vproj_ps", tag="mix")
            for e in range(NE):
                nc.tensor.matmul(
                    ps,
                    lhsT=xtv[e][:, sjb * SJ:(sjb + 1) * SJ],
                    rhs=wv_sb[:, e, :],
                    start=(e == 0), stop=(e == NE - 1),
                    skip_group_check=True)
            va = v_aug[sjb]
            va_v = va.rearrange("p (h c) -> p h c", h=4)
            nc.any.tensor_copy(
                out=va_v[:, :, 0:64],
                in_=ps.rearrange("p (h c) -> p h c", h=4))
            nc.vector.memset(va_v[:, :, 64:65], 1.0)

        pe_only = parts == "attn_pe"
        if parts in ("attn", "attn_pe"):   # bench-only: fake projection outputs
            for t in qt_sb + kt_sb:
                nc.vector.memset(t, 0.5)
            for t in v_aug:
                nc.vector.memset(t, 0.5)
        p_const = None
        if pe_only:
            p_const = psbp.tile([128, 2, SI], BF16, name="p_const", tag="pc",
                                bufs=1)
            nc.vector.memset(p_const, 0.01)

        # ---- attention + output projection, per query block of 512 ----
        for si in (range(NSI) if parts in ("all", "attn", "attn_pe") else ()):
            attn_pair = [attnsbp.tile([128, SI], BF16, name=f"ap{pr}",
                                      tag=f"ap{pr}") for pr in range(2)]
            for pr in range(2):
                nsj = (si + 1) * 4
                att = [attps.tile([65, SI], F32, name=f"att{h2}", tag="att")
                       for h2 in range(2)]
                for sjb in range(nsj):
                    a = sjb * SJ - si * SI     # >0 and <512 only on diagonal
                    amax = max(0, a)
                    qk = qkps.tile([128, 2, SI], F32, name="qk_ps", tag="qk")
                    for h2 in range(2):
                        nc.tensor.matmul(
                            qk[:, h2, amax:SI],
                            lhsT=kt_sb[pr][h2 * 64:(h2 + 1) * 64,
                                           sjb * SJ:(sjb + 1) * SJ],
                            rhs=qt_sb[pr][h2 * 64:(h2 + 1) * 64,
                                          si * SI + amax:(si + 1) * SI],
                            start=True, stop=True, skip_group_check=True)
                    if pe_only:
                        p_sb = p_const
                    else:
                        p_sb = psbp.tile([128, 2, SI], BF16, name="p_sb",
                                         tag="p")
                        nc.scalar.activation(
                            out=p_sb[:, :, amax:SI], in_=qk[:, :, amax:SI],
                            func=Exp, scale=SCALE)
                        if a >= 0:  # diagonal: tril mask on [amax, amax+128)
                            for h2 in range(2):
                                nc.vector.tensor_mul(
                                    p_sb[:, h2, amax:amax + SJ],
                                    p_sb[:, h2, amax:amax + SJ],
                                    stair_sb[:, 512:640])
                    for h2 in range(2):
                        h = pr * 2 + h2
                        nc.tensor.matmul(
                            att[h2][:, amax:SI],
                            lhsT=v_aug[sjb][:, h * 65:h * 65 + 65],
                            rhs=p_sb[:, h2, amax:SI],
                            start=(sjb == 0), stop=(sjb == nsj - 1),
                            skip_group_check=True)
                # normalize: attn_norm[d, s] = att[d, s] / att[64, s]
                for h2 in range(2):
                    rz = smallp.tile([128, SI], F32, name="rz", tag="rz")
                    nc.vector.reciprocal(rz[64:65, :], att[h2][64:65, :])
                    bc_sb = smallp.tile([64, SI], F32, name="bc_sb", tag="bcs")
                    nc.gpsimd.partition_broadcast(bc_sb, rz[64:65, :])
                    if h2 == 0:
                        nc.vector.tensor_mul(
                            attn_pair[pr][0:64, :], att[h2][0:64, :], bc_sb)
                    else:
                        tmp = smallp.tile([64, SI], BF16, name="tmp", tag="tmp")
                        nc.vector.tensor_mul(tmp, att[h2][0:64, :], bc_sb)
                        nc.sync.dma_start(out=attn_pair[pr][64:128, :], in_=tmp)
            # output projection for this query block
            for e in range(NE):
                po = mixps.tile([128, SI], F32, name="po", tag="mix")
                for t in range(2):
                    nc.tensor.matmul(
                        po,
                        lhsT=wo_sb[:, t, e * 128:(e + 1) * 128],
                        rhs=attn_pair[t],
                        start=(t == 0), stop=(t == 1), skip_group_check=True)
                po_sb = attnsbp.tile([128, SI], F32, name="po_sb", tag="po_sb")
                nc.vector.tensor_copy(out=po_sb, in_=po)
                nc.sync.dma_start(
                    out=out_t[e * 128:(e + 1) * 128, si * SI:(si + 1) * SI],
                    in_=po_sb)


def make_stair():
    p = np.arange(128)[:, None]
    x = np.arange(1024)[None, :]
    return (x >= p + 512).astype(BF16NP)


def host_prep(q, k, v, w_q, w_k, w_v, w_o):
    """Build the 8 per-core input maps."""
    stair = make_stair()
    xt = {}
    for b in range(B):
        xt[b] = [np.ascontiguousarray(a[b].T).astype(BF16NP) for a in (q, k, v)]
    maps = []
    for c in range(8):
        b, g = c // 4, c % 4
        sl = slice(g * DL, (g + 1) * DL)
        maps.append({
            "xq": xt[b][0], "xk": xt[b][1], "xv": xt[b][2],
            "wq": np.ascontiguousarray(w_q[sl].T).astype(BF16NP),
            "wk": np.ascontiguousarray(w_k[sl].T).astype(BF16NP),
            "wv": np.ascontiguousarray(w_v[sl].T).astype(BF16NP),
            "wo": np.ascontiguousarray(w_o[:, sl].T).astype(BF16NP),
            "stair": stair,
        })
    return maps


def kernel(q, k, v, mask, w_q, w_k, w_v, w_o):
    from concourse.bass_utils import run_bass_kernel_spmd

    q, k, v = (np.asarray(a, np.float32) for a in (q, k, v))
    w_q, w_k, w_v, w_o = (np.asarray(a, np.float32)
                          for a in (w_q, w_k, w_v, w_o))
    nc = build_nc()
    maps = host_prep(q, k, v, w_q, w_k, w_v, w_o)
    trace = bool(int(os.environ.get("KERNEL_TRACE", "0")))
    res = run_bass_kernel_spmd(nc, maps, list(range(8)), trace=trace)
    if trace:
        kernel.last_results = res
    out = np.empty((B, S, D), np.float32)
    for b in range(B):
        acc = res.results[b * 4]["out_t"].astype(np.float32)
        for g in range(1, 4):
            acc = acc + res.results[b * 4 + g]["out_t"]
        out[b] = acc.T
    return out
